# revision 32
# baseline (speedup 1.0000x reference)
"""Trainium2 Bass kernel for nn_DeformableConvBlock.

Sharding: pure data-parallel over batch (B=8 -> 1 batch per NeuronCore),
params replicated, no collectives.

Per-core pipeline:
  * offset/modulator conv3x3 as 9 shift-matmuls over zero-padded
    channel-major x in SBUF (TensorE, float32r = full-rate fp32).
  * PE-transpose conv output to pixel-major; tanh/sigmoid on ScalarE.
  * bilinear positions / corner weights / flat gather indices as ~35
    elementwise VectorE ops in pixel-major layout [128 px, 32 tiles, 9 taps].
  * corners (y,x) and (y,x+1) are 256 contiguous floats in a pixel-major
    x table in DRAM -> one gpsimd dma_gather row (1 KiB) per (tap, y-row);
    indices reshuffled once into the gather's 16-partition-wrapped layout.
  * bilinear combine: 2 broadcast multiplies + 3 adds per tap on VectorE.
  * PE-transpose sampled tiles to channel-major, 9-tap accumulated matmul
    with w_reg (float32r), BN+gelu fused on ScalarE, shortcut 1x1 conv with
    folded BN on TensorE, final add on VectorE.
"""

import os
import sys

for _p in ("/opt/trn_rl_repo",):
    if _p not in sys.path and os.path.isdir(_p):
        sys.path.insert(0, _p)

import numpy as np

import concourse.mybir as mybir
import concourse.tile as tile
from concourse import bacc
from concourse.bass import AP
from concourse.bass_utils import run_bass_kernel_spmd

F32 = mybir.dt.float32
F32R = mybir.dt.float32r
BF16 = mybir.dt.bfloat16
I16 = mybir.dt.int16
AL = mybir.AluOpType
AF = mybir.ActivationFunctionType

B, C, O, H, W = 8, 128, 256, 64, 64
HW = H * W            # 4096
K2 = 9
NT = HW // 128        # 32 pixel tiles of 128
NQ = 4                # spatial quarters
TPQ = NT // NQ        # 8 tiles / quarter
PXQ = 128 * TPQ       # 1024 px / quarter
TABLE_ROWS = 4104     # 1 guard row + 4096 + 7 guard rows
PAD_W = W + 2         # 66
PAD_ELEMS = (H + 2) * PAD_W  # 4356

_CACHE = {}


def _emit(nc, tc, d):
    shifts = [(dy, dx) for dy in (-1, 0, 1) for dx in (-1, 0, 1)]

    with (
        tc.tile_pool(name="const", bufs=1) as const,
        tc.tile_pool(name="offp", bufs=1) as offp,
        tc.tile_pool(name="wgt", bufs=1) as wgt,
        tc.tile_pool(name="idxp", bufs=1) as idxp,
        tc.tile_pool(name="convp", bufs=1) as convp,
        tc.tile_pool(name="scr", bufs=1) as scr,
        tc.tile_pool(name="zp", bufs=3) as zp,
        tc.tile_pool(name="mp", bufs=2) as mp,
        tc.tile_pool(name="sp", bufs=2) as sp,
        tc.tile_pool(name="sampp", bufs=1) as sampp,
        tc.tile_pool(name="outp", bufs=3) as outp,
        tc.tile_pool(name="ps_conv", bufs=1, space="PSUM") as ps_conv,
        tc.tile_pool(name="ps_trc", bufs=2, space="PSUM") as ps_trc,
        tc.tile_pool(name="ps_trs", bufs=2, space="PSUM") as ps_trs,
        tc.tile_pool(name="ps_main", bufs=2, space="PSUM") as ps_main,
    ):
        ident = const.tile([128, 128], F32, name="ident_sb")
        nc.sync.dma_start(ident[:], d["ident"].ap())
        identb = const.tile([128, 128], BF16, name="identb_sb")
        nc.sync.dma_start(identb[:], d["identb"].ap())
        wshift = const.tile([C, K2, 27], F32R, name="wshift_sb")
        nc.sync.dma_start(wshift[:], d["wshift"].ap().rearrange("s c o -> c s o"))
        bcat = const.tile([27, 1], F32, name="bcat_sb")
        nc.sync.dma_start(bcat[:], d["bcat"].ap())
        w2t = const.tile([C, K2, O], F32R, name="w2t_sb")
        nc.sync.dma_start(w2t[:], d["w2t"].ap().rearrange("k c o -> c k o"))
        wsc = const.tile([C, O], F32R, name="wsc_sb")
        nc.sync.dma_start(wsc[:], d["wsc"].ap())
        scb = const.tile([128, 6], F32, name="scb_sb")
        nc.sync.dma_start(scb[:], d["scb"].ap())
        basey = const.tile([128, NT, K2], F32, name="basey_sb")
        nc.sync.dma_start(basey[:], d["basey"].ap())
        basex = const.tile([128, NT, K2], F32, name="basex_sb")
        nc.sync.dma_start(basex[:], d["basex"].ap())

        x_pad = const.tile([C, PAD_ELEMS], F32R, name="x_pad")
        nc.vector.memset(x_pad[:].bitcast(F32), 0.0)
        x_pad3 = x_pad.rearrange("c (r w) -> c r w", w=PAD_W)
        nc.sync.dma_start(
            x_pad3[:, 1:1 + H, 1:1 + W],
            d["x"].ap().rearrange("c (h w) -> c h w", h=H),
        )

        def x_rhs(row0, nrows, dy=0, dx=0):
            return x_pad3[:, row0 + dy + 1:row0 + dy + 1 + nrows,
                          dx + 1:dx + 1 + W]

        off_pix = offp.tile([128, NT, 27], F32, name="off_pix")
        wp0 = wgt.tile([128, NT, 2, K2], BF16, name="wp0")
        wp1 = wgt.tile([128, NT, 2, K2], BF16, name="wp1")
        idxf = wgt.tile([128, NT, K2, 2], F32, name="idxf")
        idx16 = wgt.tile([128, NT, K2, 2], I16, name="idx16")
        nc.vector.memset(idx16[:], 0)
        idxw = idxp.tile([128, K2, 2, 256], I16, name="idxw")
        convout = convp.tile([27, HW], F32, name="convout")

        S9 = [128, NT, K2]
        scr_tiles = {}

        def T(nm):
            if nm not in scr_tiles:
                scr_tiles[nm] = scr.tile(S9, F32, name=nm)
            return scr_tiles[nm]

        MAGIC = 8388608.0

        def stage_a(half):
            HT = NT // 2              # tiles per half
            t0 = HT * half            # first tile
            tsl = slice(t0, t0 + HT)
            for g in range(4 * half, 4 * half + 4):
                ps = ps_conv.tile([27, 512], F32, tag="conv", name="ps_cv")
                for s, (dy, dx) in enumerate(shifts):
                    nc.tensor.matmul(
                        ps[:], lhsT=wshift[:, s, :], rhs=x_rhs(8 * g, 8, dy, dx),
                        start=(s == 0), stop=(s == 8))
                nc.scalar.activation(
                    convout[:, 512 * g:512 * (g + 1)], ps[:], AF.Identity,
                    bias=bcat[:, 0:1])
            for tt in range(t0, t0 + HT):
                pst = ps_trc.tile([128, 32], F32, tag="trc", name="ps_tc")
                nc.tensor.transpose(
                    pst[:, 0:27], convout[:, 128 * tt:128 * (tt + 1)],
                    ident[0:27, 0:27])
                nc.scalar.copy(off_pix[:, tt, :], pst[:, 0:27])

            ty, tx, mask = T("ty"), T("tx"), T("mask_t")
            nc.scalar.activation(ty[:, tsl], off_pix[:, tsl, 0:18:2], AF.Tanh)
            nc.scalar.activation(tx[:, tsl], off_pix[:, tsl, 1:18:2], AF.Tanh)
            nc.scalar.activation(mask[:, tsl], off_pix[:, tsl, 18:27], AF.Sigmoid)

            py, px = T("py"), T("px")
            nc.vector.scalar_tensor_tensor(py[:, tsl], ty[:, tsl], 3.0,
                                           basey[:, tsl], op0=AL.mult, op1=AL.add)
            nc.vector.scalar_tensor_tensor(px[:, tsl], tx[:, tsl], 3.0,
                                           basex[:, tsl], op0=AL.mult, op1=AL.add)
            wy, wx = T("wy"), T("wx")
            ty8, tx8 = T("ty8"), T("tx8")
            fy, fx = T("fy"), T("fx")
            rnd, gt = T("rnd"), T("gt_tmp")
            # floor/frac from the same rounded value t = p + 8 (exactness:
            # 2^23 magic round + fixup; frac = t - floor(t) is exact)
            for (p_, t8_, w_, f_) in ((py, ty8, wy, fy), (px, tx8, wx, fx)):
                nc.vector.tensor_scalar(t8_[:, tsl], p_[:, tsl], 8.0, None,
                                        op0=AL.add)
                nc.vector.tensor_scalar(rnd[:, tsl], t8_[:, tsl], MAGIC, -MAGIC,
                                        op0=AL.add, op1=AL.add)
                nc.vector.tensor_tensor(gt[:, tsl], rnd[:, tsl], t8_[:, tsl],
                                        op=AL.is_gt)
                nc.vector.tensor_tensor(f_[:, tsl], rnd[:, tsl], gt[:, tsl],
                                        op=AL.subtract)
                nc.vector.tensor_tensor(w_[:, tsl], t8_[:, tsl], f_[:, tsl],
                                        op=AL.subtract)
                nc.vector.tensor_scalar(f_[:, tsl], f_[:, tsl], -8.0, None,
                                        op0=AL.add)

            def validity(f, lo, hi, nm):
                a, b, v = T(nm + "a"), T(nm + "b"), T(nm + "v")
                nc.vector.tensor_scalar(a[:, tsl], f[:, tsl], lo, None,
                                        op0=AL.is_ge)
                nc.vector.tensor_scalar(b[:, tsl], f[:, tsl], hi, None,
                                        op0=AL.is_le)
                nc.vector.tensor_tensor(v[:, tsl], a[:, tsl], b[:, tsl],
                                        op=AL.mult)
                return v

            vy0 = validity(fy, 0.0, 63.0, "vy0")
            vy1 = validity(fy, -1.0, 62.0, "vy1")
            vx0 = validity(fx, 0.0, 63.0, "vx0")
            vx1 = validity(fx, -1.0, 62.0, "vx1")

            gy0, gy1, gx0, gx1, u = T("gy0"), T("gy1"), T("gx0"), T("gx1"), T("u")
            nc.vector.tensor_scalar(u[:, tsl], wy[:, tsl], -1.0, 1.0,
                                    op0=AL.mult, op1=AL.add)
            nc.vector.tensor_tensor(gy0[:, tsl], u[:, tsl], vy0[:, tsl],
                                    op=AL.mult)
            nc.vector.tensor_tensor(gy1[:, tsl], wy[:, tsl], vy1[:, tsl],
                                    op=AL.mult)
            nc.vector.tensor_scalar(u[:, tsl], wx[:, tsl], -1.0, 1.0,
                                    op0=AL.mult, op1=AL.add)
            nc.vector.tensor_tensor(gx0[:, tsl], u[:, tsl], vx0[:, tsl],
                                    op=AL.mult)
            nc.vector.tensor_tensor(gx1[:, tsl], wx[:, tsl], vx1[:, tsl],
                                    op=AL.mult)
            nc.vector.tensor_tensor(gy0[:, tsl], gy0[:, tsl], mask[:, tsl],
                                    op=AL.mult)
            nc.vector.tensor_tensor(gy1[:, tsl], gy1[:, tsl], mask[:, tsl],
                                    op=AL.mult)

            nc.vector.tensor_tensor(wp0[:, tsl, 0, :], gy0[:, tsl], gx0[:, tsl],
                                    op=AL.mult)
            nc.vector.tensor_tensor(wp0[:, tsl, 1, :], gy0[:, tsl], gx1[:, tsl],
                                    op=AL.mult)
            nc.vector.tensor_tensor(wp1[:, tsl, 0, :], gy1[:, tsl], gx0[:, tsl],
                                    op=AL.mult)
            nc.vector.tensor_tensor(wp1[:, tsl, 1, :], gy1[:, tsl], gx1[:, tsl],
                                    op=AL.mult)

            iy = T("iy")
            for corner in (0, 1):
                nc.vector.tensor_scalar(iy[:, tsl], fy[:, tsl], float(corner),
                                        0.0, op0=AL.add, op1=AL.max)
                nc.vector.tensor_scalar(iy[:, tsl], iy[:, tsl], 63.0, None,
                                        op0=AL.min)
                dst = idxf[:, tsl, :, corner]
                nc.vector.scalar_tensor_tensor(dst, iy[:, tsl], 64.0,
                                               fx[:, tsl], op0=AL.mult,
                                               op1=AL.add)
                nc.vector.tensor_scalar(dst, dst, 1.0, 0.0, op0=AL.add,
                                        op1=AL.max)
                nc.vector.tensor_scalar(dst, dst, float(TABLE_ROWS - 4), None,
                                        op0=AL.min)
            nc.vector.tensor_copy(idx16[:, tsl], idxf[:, tsl])

            # wrapped-layout scatter to DRAM (per half: t-slice) + readback
            idxs_v = d["idxs"].ap().rearrange("k c p (t hi) -> k c p t hi",
                                              hi=8)
            for hi in range(8):
                nc.sync.dma_start(
                    idxs_v[:, :, :, :, hi].transpose([2, 3, 0, 1]),
                    idx16[16 * hi:16 * (hi + 1), :, :, :],
                )
            ssl = slice(128 * half, 128 * half + 128)
            for k in range(K2):
                for cr in range(2):
                    srcv = (d["idxs"].ap()[k, cr, :, ssl]
                            .unsqueeze(0).to_broadcast([8, 16, 128]))
                    nc.sync.dma_start(idxw[:, k, cr, ssl], srcv)

        xt_gview = AP(d["xt"], 0, [[C, TABLE_ROWS - 2], [1, 2 * C]])

        def stage_b(q):
            sampled = sampp.tile([128, K2, PXQ], F32R, tag="sampled",
                                 name="sampled")
            for k in range(K2):
                z0 = zp.tile([128, TPQ, 2 * C], BF16, tag="z0", name="z0")
                z1 = zp.tile([128, TPQ, 2 * C], BF16, tag="z1", name="z1")
                for cr, z in ((0, z0), (1, z1)):
                    nc.gpsimd.dma_gather(
                        z[:], xt_gview, idxw[:, k, cr, 64 * q:64 * (q + 1)],
                        num_idxs=PXQ, num_idxs_reg=PXQ,
                        elem_size=2 * C, elem_step=C,
                    )
                m0 = mp.tile([128, TPQ, 2, C], BF16, tag="m0", name="m0")
                m1 = mp.tile([128, TPQ, 2, C], BF16, tag="m1", name="m1")
                wq0 = wp0[:, TPQ * q:TPQ * (q + 1), :, k].unsqueeze(3)
                wq1 = wp1[:, TPQ * q:TPQ * (q + 1), :, k].unsqueeze(3)
                nc.vector.tensor_tensor(
                    m0[:], z0.rearrange("p t (v c) -> p t v c", v=2),
                    wq0.to_broadcast([128, TPQ, 2, C]), op=AL.mult)
                nc.vector.tensor_tensor(
                    m1[:], z1.rearrange("p t (v c) -> p t v c", v=2),
                    wq1.to_broadcast([128, TPQ, 2, C]), op=AL.mult)
                sa = sp.tile([128, TPQ, C], BF16, tag="sa", name="sa")
                sb2 = sp.tile([128, TPQ, C], BF16, tag="sb", name="sb2")
                ss = sp.tile([128, TPQ, C], BF16, tag="ss", name="ss")
                nc.vector.tensor_tensor(sa[:], m0[:, :, 0, :], m0[:, :, 1, :],
                                        op=AL.add)
                nc.vector.tensor_tensor(sb2[:], m1[:, :, 0, :], m1[:, :, 1, :],
                                        op=AL.add)
                nc.vector.tensor_tensor(ss[:], sa[:], sb2[:], op=AL.add)
                for tt in range(TPQ):
                    pst = ps_trs.tile([128, 128], BF16, tag="trs", name="ps_ts")
                    nc.tensor.transpose(pst[:], ss[:, tt, :], identb[:])
                    nc.scalar.copy(
                        sampled[:, k, 128 * tt:128 * (tt + 1)], pst[:])

            for g2 in range(2):
                row0 = 16 * q + 8 * g2
                j0 = 512 * g2
                for h in range(2):
                    psm = ps_main.tile([128, 512], F32, tag="main", name="psm")
                    for k in range(K2):
                        nc.tensor.matmul(
                            psm[:],
                            lhsT=w2t[:, k, 128 * h:128 * (h + 1)],
                            rhs=sampled[:, k, j0:j0 + 512],
                            start=(k == 0), stop=(k == 8),
                        )
                    pss = ps_main.tile([128, 512], F32, tag="sc", name="pss",
                                       bufs=1)
                    nc.tensor.matmul(
                        pss[:], lhsT=wsc[:, 128 * h:128 * (h + 1)],
                        rhs=x_rhs(row0, 8), start=True, stop=True)
                    gel = outp.tile([128, 512], F32, tag="gel", name="gel")
                    nc.scalar.activation(gel[:], psm[:], AF.Gelu,
                                         bias=scb[:, 2 * h + 1:2 * h + 2],
                                         scale=scb[:, 2 * h:2 * h + 1])
                    ob = outp.tile([128, 512], F32, tag="ob", name="ob")
                    nc.vector.scalar_tensor_tensor(
                        ob[:], pss[:], scb[:, 4 + h:5 + h], gel[:],
                        op0=AL.add, op1=AL.add)
                    nc.sync.dma_start(
                        d["out"].ap()[128 * h:128 * (h + 1),
                                      1024 * q + j0:1024 * q + j0 + 512],
                        ob[:],
                    )

        stage_a(0)
        stage_b(0)
        stage_b(1)
        stage_a(1)
        stage_b(2)
        stage_b(3)


def _build_nc():
    nc = bacc.Bacc("TRN2", target_bir_lowering=False, debug=False)
    d = {
        "x": nc.dram_tensor("x", [C, HW], F32R, kind="ExternalInput"),
        "xt": nc.dram_tensor("xt", [TABLE_ROWS, C], BF16, kind="ExternalInput"),
        "wshift": nc.dram_tensor("wshift", [K2, C, 27], F32R, kind="ExternalInput"),
        "bcat": nc.dram_tensor("bcat", [27, 1], F32, kind="ExternalInput"),
        "w2t": nc.dram_tensor("w2t", [K2, C, O], F32R, kind="ExternalInput"),
        "wsc": nc.dram_tensor("wsc", [C, O], F32R, kind="ExternalInput"),
        "scb": nc.dram_tensor("scb", [128, 6], F32, kind="ExternalInput"),
        "basey": nc.dram_tensor("basey", [128, NT, K2], F32, kind="ExternalInput"),
        "basex": nc.dram_tensor("basex", [128, NT, K2], F32, kind="ExternalInput"),
        "ident": nc.dram_tensor("ident", [128, 128], F32, kind="ExternalInput"),
        "identb": nc.dram_tensor("identb", [128, 128], BF16, kind="ExternalInput"),
        "idxs": nc.dram_tensor("idxs", [K2, 2, 16, 256], I16, kind="Internal"),
        "out": nc.dram_tensor("out", [O, HW], F32, kind="ExternalOutput"),
    }
    with tile.TileContext(nc) as tc:
        _emit(nc, tc, d)
    nc.compile()
    return nc


def _get_nc():
    if "nc" not in _CACHE:
        _CACHE["nc"] = _build_nc()
    return _CACHE["nc"]


def _host_prep(inputs):
    f = np.float32
    x = np.ascontiguousarray(inputs["x"], dtype=f)            # [B, C, H, W]
    w_off = np.asarray(inputs["w_off"], dtype=f)              # [18, C, 3, 3]
    b_off = np.asarray(inputs["b_off"], dtype=f)
    w_mod = np.asarray(inputs["w_mod"], dtype=f)
    b_mod = np.asarray(inputs["b_mod"], dtype=f)
    w_reg = np.asarray(inputs["w_reg"], dtype=f)              # [O, C, 3, 3]
    b_reg = np.asarray(inputs["b_reg"], dtype=f)
    eps = np.float32(1e-5)

    w_cat = np.concatenate([w_off, w_mod], axis=0)            # [27, C, 3, 3]
    b_cat = np.concatenate([b_off, b_mod], axis=0).reshape(27, 1)
    wshift = np.ascontiguousarray(
        w_cat.transpose(2, 3, 1, 0).reshape(K2, C, 27))       # [9(s), C, 27]
    w2t = np.ascontiguousarray(
        w_reg.reshape(O, C, K2).transpose(2, 1, 0))           # [9(k), C, O]

    inv = np.asarray(inputs["bn_gamma"] / np.sqrt(inputs["bn_var"] + eps), f)
    bias_main = (b_reg * inv + np.asarray(inputs["bn_beta"], f)
                 - np.asarray(inputs["bn_mean"], f) * inv)
    inv_sc = np.asarray(
        inputs["sc_gamma"] / np.sqrt(inputs["sc_var"] + eps), f)
    bias_sc = (np.asarray(inputs["sc_beta"], f)
               - np.asarray(inputs["sc_mean"], f) * inv_sc)
    wsc = np.ascontiguousarray(
        (np.asarray(inputs["sc_w"], f).reshape(O, C) * inv_sc[:, None]).T)

    scb = np.zeros((128, 6), f)
    scb[:, 0] = inv[0:128];   scb[:, 1] = bias_main[0:128]
    scb[:, 2] = inv[128:256]; scb[:, 3] = bias_main[128:256]
    scb[:, 4] = bias_sc[0:128]; scb[:, 5] = bias_sc[128:256]

    j = np.arange(HW)
    ky = (np.arange(K2) // 3 - 1).astype(f)
    kx = (np.arange(K2) % 3 - 1).astype(f)
    basey = ((j // W)[:, None].astype(f) + ky[None, :]).reshape(NT, 128, K2)
    basex = ((j % W)[:, None].astype(f) + kx[None, :]).reshape(NT, 128, K2)
    basey = np.ascontiguousarray(basey.transpose(1, 0, 2))
    basex = np.ascontiguousarray(basex.transpose(1, 0, 2))

    shared = dict(wshift=wshift, bcat=b_cat, w2t=w2t, wsc=wsc, scb=scb,
                  basey=basey, basex=basex, ident=np.eye(128, dtype=f),
                  identb=np.eye(128, dtype=__import__('ml_dtypes').bfloat16))

    import ml_dtypes
    in_maps = []
    for b in range(B):
        xb = x[b].reshape(C, HW)
        xt = np.zeros((TABLE_ROWS, C), ml_dtypes.bfloat16)
        xt[1:1 + HW] = xb.T.astype(ml_dtypes.bfloat16)
        in_maps.append(dict(shared, x=np.ascontiguousarray(xb),
                            xt=np.ascontiguousarray(xt)))
    return in_maps


def kernel(**inputs):
    nc = _get_nc()
    in_maps = _host_prep(inputs)
    trace = bool(int(os.environ.get("KERNEL_TRACE", "0")))
    res = run_bass_kernel_spmd(nc, in_maps, core_ids=list(range(B)), trace=trace)
    _CACHE["last_results"] = res
    out = np.stack([res.results[b]["out"].reshape(O, H, W) for b in range(B)])
    return out.astype(np.float32)
